# revision 28
# baseline (speedup 1.0000x reference)
"""Trainium2 Bass kernel for the 2-bit-DoReFa quantized BasicBlock.

  out = conv3x3(q(bn2(conv3x3(q(bn1(x)), Wq1))), Wq2) + x
  q(h) = round(3*clip(relu(h),0,1))/3,  Wq = DoReFa-2bit(w) in {-1,-1/3,1/3,1}

Sharding: data-parallel over batch, 4 images per NeuronCore x 8 cores;
conv weights and BN parameters replicated.

Per-core kernel design:
  * Quantized activations/weights are exact small integers when scaled by 3:
    a3 in {0..3}, w3 in {-3,-1,1,3}.  Activations are stored with a +12
    offset (a' = a3+12 in {12..15}): in that fp8e4 range the lattice spacing
    is exactly 1, so the fp8 cast itself performs round-to-nearest-even --
    the separate rounding pass disappears.  Padding is memset to 12 (== 0+12)
    so the offset is spatially uniform; the resulting per-channel constant
    12*sum(W) folds into the next stage's BN bias (conv1) or the final
    rescale bias (conv2).
  * Each 3x3 conv is 9 accumulating DoubleRow 128x(2x128) matmuls per output
    tile with exact integer accumulation in fp32 PSUM; the 1/9 rescale and
    offset corrections fold into the epilogue affines.
  * The aq buffers store the two cin-halves interleaved per column
    ([p, (col, blk)]): Tile's dependency tracking is byte-interval based, so
    this keeps each conv tile's rhs read interval compact and the tile gates
    only on the quant units that wrote its rows (a stride-2 moving dim runs
    at the same 1 col/cycle PE rate, HW-verified).
  * Stage-1 quant is two DVE ops (bit-exact fp32 affine w/ folded +12 offset,
    then clamp(12,15) -> fp8 cast-round); stage-2 quant is one ACT op
    (Relu(ps*scale+bias)) + one DVE clamp; final combine is ACT
    Identity(ps*(1/9)+corr) + DVE residual add.  The vector work is split so
    the ACT queue holds only PSUM consumers (paced by the PE) and every
    engine stays far below the PE's runtime -- which also avoids the P0
    power downclock (2.0GHz) that throttled heavier-vector variants.
  * DMA: each dma_start moves ~175GB/s, so input transfers run on three
    concurrent ordered chains (x chunk halves on two, weights on one); the
    PE p-state/HAM ramp runs on a memset scratch tile during the DMA window,
    so real matmuls start ~12us in at full clock and the 1008-matmul stream
    then runs gapless at ~193ns/matmul (456 cols @ 2.36GHz).
"""
import os
from contextlib import ExitStack

import numpy as np

import concourse.bacc as bacc
import concourse.tile as tile
from concourse import mybir
from concourse.bass_utils import run_bass_kernel_spmd

F32 = mybir.dt.float32
OP = mybir.AluOpType
AF = mybir.ActivationFunctionType

N_CORES = 8
N_IMG = 4
C = 256
H = W = 56
PW = W + 1
NPIX = H * W
RT = 8
NT = H // RT
TQ = RT * PW                                   # 456
NPAD = ((PW * (H + 2) + 2 + 15) // 16) * 16    # 3312
N_CHUNK = 4
CR = H // N_CHUNK
ACT_DT = mybir.dt.float8e4
N_WARMUP = 20
AQ_INTERLEAVED = True   # aq layout [p, (col,blk)] -> compact per-tile read
                        # intervals, so conv tiles gate on just their chunks

LAST_EXEC_NS = None          # set when BASS_TRACE=1
_CACHED = {}


def _build():
    nc = bacc.Bacc("TRN2", target_bir_lowering=False, debug=False)

    x_d = nc.dram_tensor("x", [N_IMG, C, H, W], F32, kind="ExternalInput")
    w1_d = nc.dram_tensor("w1t", [128, 4608], ACT_DT, kind="ExternalInput")
    w2_d = nc.dram_tensor("w2t", [128, 4608], ACT_DT, kind="ExternalInput")
    prm_d = nc.dram_tensor("prm", [128, 10], F32, kind="ExternalInput")
    out_d = nc.dram_tensor("out", [N_IMG, C, H, W], F32, kind="ExternalOutput")

    xr = x_d.ap().rearrange("n (b k) h w -> n k b (h w)", b=2)
    outr = out_d.ap().rearrange("n (b k) h w -> n k b (h w)", b=2)

    with tile.TileContext(nc) as tc, ExitStack() as ctx:
        wpool = ctx.enter_context(tc.tile_pool(name="wpool", bufs=1))
        xpool = ctx.enter_context(tc.tile_pool(name="xpool", bufs=4))
        aqpool = ctx.enter_context(tc.tile_pool(name="aqpool", bufs=1))
        t1pool = ctx.enter_context(tc.tile_pool(name="t1pool", bufs=3))
        t2pool = ctx.enter_context(tc.tile_pool(name="t2pool", bufs=6))
        t3pool = ctx.enter_context(tc.tile_pool(name="t3pool", bufs=6))
        pspool = ctx.enter_context(tc.tile_pool(name="pspool", bufs=7,
                                                space="PSUM"))
        wupool = ctx.enter_context(tc.tile_pool(name="wupool", bufs=1,
                                                space="PSUM"))

        # PE warmup scratch: memset only (no DMA dependency), so the
        # p-state/HAM ramp starts as early as possible.
        wu_src = aqpool.tile([128, 2, 512], ACT_DT)
        nc.gpsimd.memset(wu_src[:], 1.0)

        prm = wpool.tile([128, 10], F32)
        nc.sync.dma_start(prm[:], prm_d.ap())
        w1_sb = wpool.tile([128, 4608], ACT_DT)
        w2_sb = wpool.tile([128, 4608], ACT_DT)

        # Fixed ping-pong padded activation buffers; borders memset to 12
        # (= quant offset for a=0) once -- interior writes never touch them.
        # AQ_INTERLEAVED stores the two cin-halves interleaved per column
        # ([p, (col, blk)]): a conv tile's rhs then reads a compact byte
        # interval, so its dependencies cover only the rows it touches.
        aq1s, aq2s = [], []
        for i in range(2):
            if AQ_INTERLEAVED:
                a1 = aqpool.tile([128, 2 * NPAD], ACT_DT, name=f"aq1_{i}", tag=f"aq1_{i}")
                a2 = aqpool.tile([128, 2 * NPAD], ACT_DT, name=f"aq2_{i}", tag=f"aq2_{i}")
            else:
                a1 = aqpool.tile([128, 2, NPAD], ACT_DT, name=f"aq1_{i}", tag=f"aq1_{i}")
                a2 = aqpool.tile([128, 2, NPAD], ACT_DT, name=f"aq2_{i}", tag=f"aq2_{i}")
            aq1s.append(a1)
            aq2s.append(a2)
            for a in (a1, a2):
                if AQ_INTERLEAVED:
                    nc.gpsimd.memset(a[:, 0:2 * (PW + 1)], 12.0)
                    mid = a[:, 2 * (PW + W + 1): 2 * (PW + W + 1 + (H - 1) * PW)]
                    mid3 = mid.rearrange("p (r c) -> p r c", c=2 * PW)
                    nc.gpsimd.memset(mid3[:, :, 0:2 * (PW - W)], 12.0)
                    nc.gpsimd.memset(a[:, 2 * (H * PW + W + 1): 2 * NPAD], 12.0)
                else:
                    for blk in range(2):
                        nc.gpsimd.memset(a[:, blk, 0:PW + 1], 12.0)
                        mid = a[:, blk, PW + W + 1: PW + W + 1 + (H - 1) * PW]
                        mid3 = mid.rearrange("p (r c) -> p r c", c=PW)[:, :, 0:PW - W]
                        nc.gpsimd.memset(mid3, 12.0)
                        nc.gpsimd.memset(a[:, blk, H * PW + W + 1: NPAD], 12.0)

        def aq_dst(aq, blk, rows, y0):
            """Interior write view [p, rows, W] for quant output."""
            lo = (y0 + 1) * PW + 1
            if AQ_INTERLEAVED:
                A = aq[:].rearrange("p (n j) -> p n j", j=2)
                d = A[:, lo: lo + rows * PW, blk]
            else:
                d = aq[:, blk, lo: lo + rows * PW]
            return d.rearrange("p (r c) -> p r c", c=PW)[:, :, 0:W]

        def aq_rhs(aq, t, tap):
            """Matmul rhs [p, 2, TQ] for tile t, tap."""
            ky, kx = divmod(tap, 3)
            off = t * TQ + ky * PW + kx
            if AQ_INTERLEAVED:
                return aq[:, 2 * off: 2 * (off + TQ)].rearrange(
                    "p (n j) -> p j n", j=2)
            return aq[:, :, off: off + TQ]

        # --- DMA ordering: a single dma_start tops out at ~175GB/s, so run
        # three concurrent chains (two for image chunks, one for weights),
        # each internally ordered so early data still lands first.
        last_dma = [None, None, None]

        def chain(cidx, inst):
            # add_dep_helper(dependent, prerequisite): inst waits chain tail.
            if last_dma[cidx] is not None:
                tile.add_dep_helper(inst.ins, last_dma[cidx].ins, sync=True,
                                    reason="serialize DMA chain")
            last_dma[cidx] = inst
            return inst

        x_sbs = [None] * N_IMG

        def x_alloc(img):
            x_sb = xpool.tile([128, 2, NPIX], F32, tag="x", name=f"x_{img}")
            x_sbs[img] = x_sb

        def x_chunk_dma(img, ch):
            base = ch * CR * W
            q = CR * W // 2
            for h in range(2):
                sl = slice(base + h * q, base + (h + 1) * q)
                chain(h, nc.sync.dma_start(x_sbs[img][:, :, sl],
                                           xr[img][:, :, sl]))

        def stage1_dma(img):
            x_alloc(img)
            for ch in range(N_CHUNK):
                x_chunk_dma(img, ch)

        def quant1_stage(src_ap, aq, blk, s_col, c_col, rows, y0):
            """Stage-1 quant, DVE only (bit-exact fp32 affine):
            a' = cast_fp8(min(max(src*s + coff, 12), 15)), coff = 3*bias+12;
            the fp8 cast in [12,15] rounds half-even == jnp.round."""
            t = t1pool.tile([128, rows * W], F32, tag="q1tmp")
            t3 = t[:].rearrange("p (r c) -> p r c", c=W)
            nc.vector.tensor_scalar(t3, src_ap, prm[:, s_col:s_col + 1],
                                    prm[:, c_col:c_col + 1], OP.mult, OP.add)
            nc.vector.tensor_scalar(aq_dst(aq, blk, rows, y0), t3,
                                    12.0, 15.0, OP.max, OP.min)

        def quant2_stage(src_ap, aq, blk, s_col, c_col, rows, y0):
            """Stage-2 quant: ACT affine+Relu (PSUM consumer) + DVE clamp."""
            t = t2pool.tile([128, rows * W], F32, tag="q2tmp")
            t3 = t[:].rearrange("p (r c) -> p r c", c=W)
            nc.scalar.activation(t3, src_ap, AF.Relu,
                                 scale=prm[:, s_col:s_col + 1],
                                 bias=prm[:, c_col:c_col + 1])
            nc.vector.tensor_scalar(aq_dst(aq, blk, rows, y0), t3,
                                    3.0, 12.0, OP.min, OP.add)

        def conv_tile(aq, w_sb, t, cb):
            ps = pspool.tile([128, TQ], F32, tag="ps")
            w4 = w_sb[:].rearrange("p (t j m) -> p t j m", t=9, j=2)
            for tap in range(9):
                lhsT = w4[:, tap, :, cb * 128:cb * 128 + 128]
                nc.tensor.matmul(ps[:], lhsT, aq_rhs(aq, t, tap),
                                 perf_mode=mybir.MatmulPerfMode.DoubleRow,
                                 start=(tap == 0), stop=(tap == 8))
            return ps

        def stage1_units(img):
            """Quant thunks for image img, one per (chunk, blk)."""
            aq1 = aq1s[img % 2]
            x_sb = x_sbs[img]

            def make(ch, blk):
                def run():
                    sl = slice(ch * CR * W, (ch + 1) * CR * W)
                    src = x_sb[:, blk, sl].rearrange("p (r c) -> p r c", c=W)
                    quant1_stage(src, aq1, blk, 0 + blk, 2 + blk, CR, ch * CR)
                return run
            return [make(ch, blk) for ch in range(N_CHUNK) for blk in range(2)]

        def conv1_tile(img, t, cb):
            aq1, aq2 = aq1s[img % 2], aq2s[img % 2]
            ps = conv_tile(aq1, w1_sb, t, cb)
            psv = ps[:].rearrange("p (r c) -> p r c", c=PW)[:, :, 0:W]
            quant2_stage(psv, aq2, cb, 4 + cb, 6 + cb, RT, t * RT)

        def conv2_tile(img, t, cb, split_epi=False):
            aq2, x_sb = aq2s[img % 2], x_sbs[img]
            ps = conv_tile(aq2, w2_sb, t, cb)
            psa = ps[:].rearrange("p (r c) -> p r c", c=PW)
            # The very last tiles split their epilogue in half-row batches
            # to shorten the post-final-matmul critical chain.
            nh = 2 if split_epi else 1
            rh = RT // nh
            for h in range(nh):
                psv = psa[:, h * rh:(h + 1) * rh, 0:W]
                tt = t3pool.tile([128, rh * W], F32, tag="ot",
                                 name=f"ot_{img}_{t}_{cb}_{h}")
                tt3 = tt[:].rearrange("p (r c) -> p r c", c=W)
                nc.scalar.activation(tt3, psv, AF.Identity, scale=1.0 / 9.0,
                                     bias=prm[:, 8 + cb:9 + cb])
                lo = t * RT * W + h * rh * W
                res = x_sb[:, cb, lo: lo + rh * W]
                res3 = res.rearrange("p (r c) -> p r c", c=W)
                nc.vector.tensor_tensor(res3, tt3, res3, OP.add)
                nc.sync.dma_start(outr[img][:, cb, lo: lo + rh * W], res)

        def conv1_img(img, interleave=()):
            # Spread the next image's stage-1 units between conv1 tiles so
            # the DVE/ACT queues never head-of-line block the conv epilogues.
            inter = list(interleave)
            for t in range(NT):
                for cb in range(2):
                    conv1_tile(img, t, cb)
                for _ in range(2):
                    if inter:
                        inter.pop(0)()
            for f in inter:
                f()

        def conv2_img(img, last=False):
            for t in range(NT):
                for cb in range(2):
                    conv2_tile(img, t, cb,
                               split_epi=(last and t == NT - 1))

        # --- startup: x0 chunk0 first, then w1 (needed ~12us in), then the
        # rest of x0, then w2; warmup matmuls run during the DMA window.
        x_alloc(0)
        for ch in range(N_CHUNK):
            x_chunk_dma(0, ch)
        chain(2, nc.sync.dma_start(w1_sb[:], w1_d.ap()))
        chain(2, nc.sync.dma_start(w2_sb[:], w2_d.ap()))

        wu_ps = wupool.tile([128, TQ], F32)
        for i in range(N_WARMUP):
            nc.tensor.matmul(wu_ps[:], wu_src[:, :, 0:128],
                             wu_src[:, :, 0:TQ],
                             perf_mode=mybir.MatmulPerfMode.DoubleRow,
                             start=(i == 0), stop=(i == N_WARMUP - 1))

        for f in stage1_units(0):
            f()
        if N_IMG > 1:
            stage1_dma(1)
            conv1_img(0, interleave=stage1_units(1))
        else:
            conv1_img(0)
        for img in range(1, N_IMG):
            nxt = ()
            if img + 1 < N_IMG:
                stage1_dma(img + 1)
                nxt = stage1_units(img + 1)
            conv1_img(img, interleave=nxt)
            conv2_img(img - 1)
        conv2_img(N_IMG - 1, last=True)

    nc.compile()
    return nc


def _host_prep(w1, w2, g1, b1, m1, v1, g2, b2, m2, v2):
    """BN folds + DoReFa weight quantization, replicating the reference's
    fp32 op sequence exactly (jax CPU), then weight layout transforms."""
    import jax
    import jax.numpy as jnp
    import ml_dtypes

    cpu = jax.local_devices(backend="cpu")[0]
    with jax.default_device(cpu):
        eps = jnp.float32(1e-5)
        inv1 = g1 / jnp.sqrt(v1 + eps)
        bias1 = b1 - m1 * inv1
        inv2 = g2 / jnp.sqrt(v2 + eps)
        bias2 = b2 - m2 * inv2

        def wq3(w):
            wt = jnp.tanh(w)
            wn = wt / (2.0 * jnp.max(jnp.abs(wt))) + 0.5
            return 2.0 * jnp.round(wn * 3.0) - 3.0   # exact ints {-3,-1,1,3}

        wq1 = np.asarray(wq3(jnp.asarray(w1)), dtype=np.float32)
        wq2 = np.asarray(wq3(jnp.asarray(w2)), dtype=np.float32)
        inv1, bias1, inv2, bias2 = (np.asarray(a, dtype=np.float32)
                                    for a in (inv1, bias1, inv2, bias2))

    S1 = wq1.reshape(256, -1).sum(axis=1).astype(np.float32)
    S2 = wq2.reshape(256, -1).sum(axis=1).astype(np.float32)

    s1 = 3.0 * inv1
    c1 = 3.0 * bias1 + 12.0      # stage-1 offset folded into the affine
    s2 = inv2 / np.float32(3.0)
    c2 = 3.0 * bias2 - 4.0 * S1 * inv2
    corr2 = -(np.float32(4.0) / np.float32(3.0)) * S2

    def wlayout(wq):
        # [cout, cin, ky, kx] -> [k(128), tap(9), blk(2), cout(256)]
        a = wq.reshape(256, 2, 128, 9)                     # cout, blk, k, tap
        return np.ascontiguousarray(np.transpose(a, (2, 3, 1, 0))
                                    .reshape(128, 4608)
                                    ).astype(ml_dtypes.float8_e4m3)

    prm = np.zeros((128, 10), np.float32)
    for col, v in enumerate((s1, c1, s2, c2, corr2)):
        prm[:, 2 * col] = v[0:128]
        prm[:, 2 * col + 1] = v[128:256]

    return {"w1t": wlayout(wq1), "w2t": wlayout(wq2), "prm": prm}


def kernel(x, w1, w2, g1, b1, m1, v1, g2, b2, m2, v2):
    global LAST_EXEC_NS
    x = np.asarray(x, dtype=np.float32)

    if "nc" not in _CACHED:
        _CACHED["nc"] = _build()
    nc = _CACHED["nc"]

    shared = _host_prep(w1, w2, g1, b1, m1, v1, g2, b2, m2, v2)
    in_maps = []
    for c in range(N_CORES):
        m = dict(shared)
        m["x"] = x[N_IMG * c:N_IMG * (c + 1)]
        in_maps.append(m)

    trace = bool(int(os.environ.get("BASS_TRACE", "0")))
    res = run_bass_kernel_spmd(nc, in_maps, core_ids=list(range(N_CORES)),
                               trace=trace)
    LAST_EXEC_NS = res.exec_time_ns
    return np.concatenate([res.results[c]["out"] for c in range(N_CORES)],
                          axis=0)
